# revision 23
# baseline (speedup 1.0000x reference)
"""Trainium2 Bass kernel for nn_BoundaryHRM (2-layer packed GRU + ACT controller + head).

Sharding: data-parallel over batch across 8 cores (4 sequences/core); weights
replicated; the time scans run locally per core in a transposed
[feature-on-partitions, lane-on-free] layout so the per-step gate math is cheap.

Self-contained: hardcodes all shapes; host-side prep only transposes/casts/
packs weights and lays out per-core inputs.
"""
import os
import numpy as np
import ml_dtypes

import concourse.bass as bass
import concourse.bacc as bacc
import concourse.tile as tile
from concourse import mybir
from concourse.bass_utils import run_bass_kernel_spmd
from concourse.masks import make_identity

# model dims
B, T, E, H, C, NCLS, V = 32, 512, 256, 512, 256, 256, 32000
NCORES = 8
BL = B // NCORES          # lanes per core = 4
NT = T * BL               # tokens per core = 2048
G = 3 * H                 # 1536
GC = G // 128             # 12 gate chunks
HC = H // 128             # 4 h chunks
EC = E // 128             # 2 e chunks
CC = C // 128             # 2 ctrl chunks
CGC = 3 * C // 128        # 6 ctrl gate chunks

F32 = mybir.dt.float32
BF16 = mybir.dt.bfloat16
I32 = mybir.dt.int32
AF = mybir.ActivationFunctionType
OP = mybir.AluOpType
ds = bass.ds

SIG_NEG20 = float(np.float32(1.0 / (1.0 + np.exp(np.float64(20.0)))))

U = 8                     # scan unroll per For_i body


def bcf(ap2d, reps):
    """[128, n] slice -> [128, reps, n] free-broadcast view."""
    return ap2d.unsqueeze(1).broadcast_to([ap2d.shape[0], reps, ap2d.shape[-1]])


def build(scalars, t_steps=T, debug=False):
    """Build the Bacc program. scalars: dict with bs_b, gate_b floats."""
    nc = bacc.Bacc("TRN2", target_bir_lowering=False, debug=False,
                   num_devices=NCORES)

    def din(name, shape, dt):
        return nc.dram_tensor(name, shape, dt, kind="ExternalInput").ap()

    ids_d = din("ids", [128, NT // 128], I32)          # ids[p,g] = token g*128+p
    lens_d = din("lens", [128, BL], F32)               # replicated lengths
    emb_d = din("emb", [V, E], F32)
    wih0_d = din("wih0T", [128, EC * G], BF16)
    whh0_d = din("whh0T", [128, HC * G], BF16)
    wih1_d = din("wih1T", [128, HC * G], BF16)
    whh1_d = din("whh1T", [128, HC * G], BF16)
    bias0_d = din("bias0", [128, GC], F32)             # b_ih (+b_hh for rz)
    bias1_d = din("bias1", [128, GC], F32)
    bhn0r_d = din("bhn0r", [1, H], BF16)               # b_hh n-gate rows
    bhn1r_d = din("bhn1r", [1, H], BF16)
    bsw_d = din("bswT", [128, HC * 128], BF16)         # bs_w replicated M=128
    spw_d = din("spwT", [128, HC * C], BF16)
    cwih_d = din("cwihT", [128, CC * 768], BF16)
    cwhh_d = din("cwhhT", [128, CC * 768], BF16)
    cbias_d = din("cbias", [1, 1024], BF16)            # [biasA(768) | biasB(256)]
    cow_d = din("cowT", [128, CC * H], BF16)
    cob_d = din("cob", [128, HC], F32)
    gatew_d = din("gatewT", [128, (HC + CC) * 128], BF16)  # replicated M=128
    hw1_d = din("hw1T", [128, HC * H], BF16)
    hb1_d = din("hb1", [128, HC], F32)
    hw2_d = din("hw2T", [128, HC * NCLS], BF16)
    hb2_d = din("hb2", [128, NCLS // 128], F32)

    out_d = nc.dram_tensor("out", [BL, NCLS], F32, kind="ExternalOutput").ap()
    if debug:
        dbg_hT_d = nc.dram_tensor("dbg_hT", [128, 16], F32, kind="ExternalOutput").ap()
        dbg_p_d = nc.dram_tensor("dbg_p", [128, NT], F32, kind="ExternalOutput").ap()
        dbg_ctrl_d = nc.dram_tensor("dbg_ctrl", [128, 8], F32, kind="ExternalOutput").ap()
        dbg_o0_d = nc.dram_tensor("dbg_o0", [128, 16], F32, kind="ExternalOutput").ap()
        dbg_ha_d = nc.dram_tensor("dbg_ha", [128, BL], F32, kind="ExternalOutput").ap()

    from contextlib import ExitStack
    with tile.TileContext(nc) as tc, ExitStack() as ctx:
        pp = ctx.enter_context(tc.tile_pool(name="persist", bufs=1))

        # ---- persistent SBUF state ----
        GI = pp.tile([128, GC, NT], BF16, tag="GI")
        out0 = pp.tile([128, T, 4, BL], BF16, tag="out0")
        gru = pp.tile([128, T, 4, BL], BF16, tag="gru")
        h16 = pp.tile([128, HC, BL], BF16, tag="h16")
        valid = pp.tile([128, NT], F32, tag="valid")
        p128 = pp.tile([128, NT], F32, tag="p128")
        ctrl16 = pp.tile([128, CC, BL], BF16, tag="ctrl16")
        seg = pp.tile([128, HC, BL], F32, tag="seg")
        ha = pp.tile([128, BL], F32, tag="ha")
        zero_hb = pp.tile([128, HC, BL], F32, tag="zero_hb")
        ones16 = pp.tile([1, BL], BF16, tag="ones16")
        bsb_ap = pp.tile([128, 1], F32, tag="bsb_ap")
        gateb_ap = pp.tile([128, 1], F32, tag="gateb_ap")

        # ---- persistent weights ----
        wih0 = pp.tile([128, EC * G], BF16, tag="wih0")
        whh0 = pp.tile([128, HC * G], BF16, tag="whh0")
        wih1 = pp.tile([128, HC * G], BF16, tag="wih1")
        whh1 = pp.tile([128, HC * G], BF16, tag="whh1")
        bias0 = pp.tile([128, GC], F32, tag="bias0")
        bias1 = pp.tile([128, GC], F32, tag="bias1")
        bsw = pp.tile([128, HC * 128], BF16, tag="bsw")
        spw = pp.tile([128, HC * C], BF16, tag="spw")
        cwih = pp.tile([128, CC * 768], BF16, tag="cwih")
        cwhh = pp.tile([128, CC * 768], BF16, tag="cwhh")
        cbias = pp.tile([1, 1024], BF16, tag="cbias")
        cow = pp.tile([128, CC * H], BF16, tag="cow")
        cob = pp.tile([128, HC], F32, tag="cob")
        gatew = pp.tile([128, (HC + CC) * 128], BF16, tag="gatew")
        hw1 = pp.tile([128, HC * H], BF16, tag="hw1")
        hb1 = pp.tile([128, HC], F32, tag="hb1")
        hw2 = pp.tile([128, HC * NCLS], BF16, tag="hw2")
        hb2 = pp.tile([128, NCLS // 128], F32, tag="hb2")
        lens = pp.tile([128, BL], F32, tag="lens")
        ids_t = pp.tile([128, NT // 128], I32, tag="ids")
        bhn16 = pp.tile([1, H], BF16, tag="bhn16")
        bhn16b = pp.tile([1, H], BF16, tag="bhn16b")

        for tile_, dram in [(wih0, wih0_d), (whh0, whh0_d), (wih1, wih1_d),
                            (whh1, whh1_d), (bias0, bias0_d), (bias1, bias1_d),
                            (bsw, bsw_d),
                            (spw, spw_d), (cwih, cwih_d), (cwhh, cwhh_d),
                            (cbias, cbias_d), (cow, cow_d), (cob, cob_d),
                            (gatew, gatew_d), (hw1, hw1_d), (hb1, hb1_d),
                            (hw2, hw2_d), (hb2, hb2_d), (lens, lens_d),
                            (ids_t, ids_d), (bhn16, bhn0r_d), (bhn16b, bhn1r_d)]:
            nc.sync.dma_start(tile_[:], dram[:])

        nc.vector.memset(zero_hb[:], 0.0)
        nc.vector.memset(ones16[:], 1.0)
        nc.vector.memset(bsb_ap[:], float(scalars["bs_b"]))
        nc.vector.memset(gateb_ap[:], float(scalars["gate_b"]))
        nc.vector.memset(h16[:], 0.0)
        nc.vector.memset(ctrl16[:], 0.0)
        nc.vector.memset(out0[:], 0.0)
        nc.vector.memset(gru[:], 0.0)
        nc.vector.memset(seg[:], 0.0)
        nc.vector.memset(ha[:], 0.0)

        # ---- phase A: valid mask ----
        with tc.tile_pool(name="maskp", bufs=1) as mp:
            it32 = mp.tile([128, NT], I32)
            itf = mp.tile([128, NT], F32)
            nc.gpsimd.iota(it32[:], pattern=[[1, T], [0, BL]], base=0,
                           channel_multiplier=0)
            nc.vector.tensor_copy(itf[:], it32[:])
            nc.vector.tensor_tensor(
                out=valid[:].rearrange("p (t b) -> p t b", b=BL),
                in0=itf[:].rearrange("p (t b) -> p t b", b=BL),
                in1=bcf(lens[:], T),
                op=OP.is_lt)

        # ---- phase B+C: embedding gather, transpose, GI0 ----
        with tc.tile_pool(name="xp", bufs=1) as xp, \
             tc.tile_pool(name="gp", bufs=3) as gp, \
             tc.tile_pool(name="tp", bufs=4, space="PSUM") as tp:
            xT = xp.tile([128, EC, NT], BF16)
            ident = xp.tile([128, 128], F32)
            make_identity(nc, ident[:])
            for g in range(NT // 128):
                xg = gp.tile([128, E], F32, tag="xg")
                nc.gpsimd.indirect_dma_start(
                    out=xg[:], out_offset=None, in_=emb_d[:],
                    in_offset=bass.IndirectOffsetOnAxis(ap=ids_t[:, g:g + 1], axis=0))
                for k in range(EC):
                    ps = tp.tile([128, 128], F32, tag="tps")
                    nc.tensor.transpose(ps[:], xg[:, k * 128:(k + 1) * 128], ident[:])
                    nc.vector.tensor_copy(xT[:, k, g * 128:(g + 1) * 128], ps[:])

            for gc in range(GC):
                for nb in range(NT // 512):
                    ps = tp.tile([128, 512], F32, tag="gips")
                    for k in range(EC):
                        nc.tensor.matmul(
                            ps[:], wih0[:, k * G + gc * 128: k * G + (gc + 1) * 128],
                            xT[:, k, nb * 512:(nb + 1) * 512],
                            start=(k == 0), stop=(k == EC - 1))
                    nc.vector.tensor_scalar_add(
                        GI[:, gc, nb * 512:(nb + 1) * 512], ps[:], bias0[:, gc:gc + 1])

        # ---- phase D: layer-0 scan ----
        def gru_scan(whh, bhn, dst, layer):
            with tc.tile_pool(name=f"sc{layer}", bufs=3) as sp, \
                 tc.tile_pool(name=f"scp{layer}", bufs=2, space="PSUM") as spp:
                with tc.For_i(0, t_steps // U) as iv:
                    for u in range(U):
                        t4 = iv * (U * BL) + u * BL
                        prz = spp.tile([128, 8, BL], F32, tag="prz")
                        pn = spp.tile([128, HC, BL], F32, tag="pn")
                        for gc in range(8):
                            for k in range(HC):
                                nc.tensor.matmul(
                                    prz[:, gc, :],
                                    whh[:, k * G + gc * 128: k * G + (gc + 1) * 128],
                                    h16[:, k, :], start=(k == 0), stop=(k == HC - 1))
                        for gc in range(8, GC):
                            c = gc - 8
                            nc.tensor.matmul(
                                pn[:, c, :], bhn[:, c * 128:(c + 1) * 128],
                                ones16[:], start=True, stop=False)
                            for k in range(HC):
                                nc.tensor.matmul(
                                    pn[:, c, :],
                                    whh[:, k * G + gc * 128: k * G + (gc + 1) * 128],
                                    h16[:, k, :], start=False, stop=(k == HC - 1))
                        rzs = sp.tile([128, 8, BL], F32, tag="rzs")
                        nc.vector.tensor_tensor(out=rzs[:], in0=prz[:],
                                                in1=GI[:, 0:8, ds(t4, BL)], op=OP.add)
                        rz = sp.tile([128, 8, BL], F32, tag="rz")
                        nc.scalar.activation(rz[:], rzs[:], AF.Sigmoid)
                        npre = sp.tile([128, HC, BL], F32, tag="npre")
                        nc.vector.tensor_tensor(out=npre[:], in0=pn[:],
                                                in1=rz[:, 0:4, :], op=OP.mult)
                        npre2 = sp.tile([128, HC, BL], F32, tag="npre2")
                        nc.vector.tensor_tensor(out=npre2[:], in0=npre[:],
                                                in1=GI[:, 8:12, ds(t4, BL)], op=OP.add)
                        nt_ = sp.tile([128, HC, BL], F32, tag="nt")
                        nc.scalar.activation(nt_[:], npre2[:], AF.Tanh)
                        d_ = sp.tile([128, HC, BL], F32, tag="d")
                        nc.vector.tensor_tensor(out=d_[:], in0=nt_[:], in1=h16[:],
                                                op=OP.subtract)
                        zd = sp.tile([128, HC, BL], F32, tag="zd")
                        nc.vector.tensor_tensor(out=zd[:], in0=rz[:, 4:8, :], in1=d_[:],
                                                op=OP.mult)
                        hnew = sp.tile([128, HC, BL], BF16, tag="hnew")
                        nc.vector.tensor_tensor(out=hnew[:], in0=nt_[:], in1=zd[:],
                                                op=OP.subtract)
                        mb = sp.tile([128, HC, BL], mybir.dt.uint8, tag="mb")
                        mbf = sp.tile([128, HC, BL], BF16, tag="mbf")
                        nc.vector.tensor_copy(
                            mb[:], valid[:, ds(t4, BL)].unsqueeze(1).broadcast_to(
                                [128, HC, BL]))
                        nc.vector.tensor_copy(mbf[:], mb[:])
                        nc.vector.copy_predicated(out=h16[:], mask=mb[:],
                                                  data=hnew[:])
                        tloc = iv * U + u
                        nc.vector.tensor_tensor(
                            out=dst[:, ds(tloc, 1), :, :].squeeze(1),
                            in0=h16[:], in1=mbf[:], op=OP.mult)

        gru_scan(whh0, bhn16, out0, 0)

        # ---- phase E: GI1 = w_ih1 @ out0 + bias1 ----
        with tc.tile_pool(name="gi1p", bufs=4, space="PSUM") as tp1:
            for gc in range(GC):
                for nb in range(NT // 512):
                    ps = tp1.tile([128, 512], F32, tag="gi1ps")
                    for k in range(HC):
                        nc.tensor.matmul(
                            ps[:], wih1[:, k * G + gc * 128: k * G + (gc + 1) * 128],
                            out0[:, nb * 128:(nb + 1) * 128, k, :],
                            start=(k == 0), stop=(k == HC - 1))
                    nc.vector.tensor_scalar_add(
                        GI[:, gc, nb * 512:(nb + 1) * 512], ps[:], bias1[:, gc:gc + 1])

        # ---- phase F: layer-1 scan ----
        nc.vector.memset(h16[:], 0.0)
        gru_scan(whh1, bhn16b, gru, 1)

        # ---- phase G: halt probabilities ----
        with tc.tile_pool(name="hp", bufs=2) as hpool, \
             tc.tile_pool(name="hpp", bufs=2, space="PSUM") as hpp:
            for nb in range(NT // 512):
                ps = hpp.tile([128, 512], F32, tag="hps")
                for k in range(HC):
                    nc.tensor.matmul(
                        ps[:], bsw[:, k * 128:(k + 1) * 128],
                        gru[:, nb * 128:(nb + 1) * 128, k, :],
                        start=(k == 0), stop=(k == HC - 1))
                sl = hpool.tile([128, 512], F32, tag="sl")
                nc.scalar.activation(sl[:], ps[:], AF.Sigmoid, bias=bsb_ap[:])
                pt = hpool.tile([128, 512], F32, tag="pt")
                nc.vector.scalar_tensor_tensor(
                    out=pt[:], in0=sl[:], scalar=SIG_NEG20,
                    in1=valid[:, nb * 512:(nb + 1) * 512],
                    op0=OP.subtract, op1=OP.mult)
                nc.vector.tensor_single_scalar(
                    out=p128[:, nb * 512:(nb + 1) * 512], in_=pt[:],
                    scalar=SIG_NEG20, op=OP.add)

        # ---- phase H: ACT controller scan ----
        def ctrl_update_step(sp, spp, srep16):
            """Emit ctrl-update math given seg_repr.T bf16 [128, CC, BL].
            Returns cnew (f32 [128, CC, BL])."""
            pseg = spp.tile([128, CC, BL], F32, tag="pseg")
            for m in range(CC):
                for k in range(HC):
                    nc.tensor.matmul(
                        pseg[:, m, :], spw[:, k * C + m * 128: k * C + (m + 1) * 128],
                        srep16[:, k, :], start=(k == 0), stop=(k == HC - 1))
            segin16 = sp.tile([128, CC, BL], BF16, tag="segin16")
            nc.vector.tensor_copy(segin16[:], pseg[:])
            pA = spp.tile([128, CGC, BL], F32, tag="pA")
            pB = spp.tile([128, CC, BL], F32, tag="pB")
            # bias rows: cbias [1, 1024] = [biasA(768) | biasB(256)].
            # Per-gc groups are emitted contiguously (open -> close) so PSUM
            # accumulation groups never interleave within a tile.
            for gc in range(CGC):
                nc.tensor.matmul(pA[:, gc, :], cbias[:, gc * 128:(gc + 1) * 128],
                                 ones16[:], start=True, stop=False)
                if gc < 4:  # r,z: add gh
                    for k in range(CC):
                        nc.tensor.matmul(
                            pA[:, gc, :],
                            cwhh[:, k * 768 + gc * 128: k * 768 + (gc + 1) * 128],
                            ctrl16[:, k, :], start=False, stop=False)
                for k in range(CC):
                    nc.tensor.matmul(
                        pA[:, gc, :],
                        cwih[:, k * 768 + gc * 128: k * 768 + (gc + 1) * 128],
                        segin16[:, k, :], start=False, stop=(k == CC - 1))
            for m in range(CC):
                gc = 4 + m
                nc.tensor.matmul(pB[:, m, :],
                                 cbias[:, 768 + m * 128: 768 + (m + 1) * 128],
                                 ones16[:], start=True, stop=False)
                for k in range(CC):
                    nc.tensor.matmul(
                        pB[:, m, :],
                        cwhh[:, k * 768 + gc * 128: k * 768 + (gc + 1) * 128],
                        ctrl16[:, k, :], start=False, stop=(k == CC - 1))
            rzc = sp.tile([128, 4, BL], F32, tag="rzc")
            nc.scalar.activation(rzc[:], pA[:, 0:4, :], AF.Sigmoid)
            npc = sp.tile([128, CC, BL], F32, tag="npc")
            nc.vector.tensor_tensor(out=npc[:], in0=pB[:], in1=rzc[:, 0:2, :],
                                    op=OP.mult)
            npc2 = sp.tile([128, CC, BL], F32, tag="npc2")
            nc.vector.tensor_tensor(out=npc2[:], in0=npc[:], in1=pA[:, 4:6, :],
                                    op=OP.add)
            ntc = sp.tile([128, CC, BL], F32, tag="ntc")
            nc.scalar.activation(ntc[:], npc2[:], AF.Tanh)
            dc = sp.tile([128, CC, BL], F32, tag="dc")
            nc.vector.tensor_tensor(out=dc[:], in0=ntc[:], in1=ctrl16[:],
                                    op=OP.subtract)
            zdc = sp.tile([128, CC, BL], F32, tag="zdc")
            nc.vector.tensor_tensor(out=zdc[:], in0=rzc[:, 2:4, :], in1=dc[:],
                                    op=OP.mult)
            cnew = sp.tile([128, CC, BL], BF16, tag="cnew")
            nc.vector.tensor_tensor(out=cnew[:], in0=ntc[:], in1=zdc[:],
                                    op=OP.subtract)
            return cnew

        def seg_repr16(sp):
            ham = sp.tile([128, BL], F32, tag="ham")
            nc.vector.tensor_single_scalar(out=ham[:], in_=ha[:], scalar=1e-6,
                                           op=OP.max)
            rec = sp.tile([128, BL], F32, tag="rec")
            nc.vector.reciprocal(rec[:], ham[:])
            srep16 = sp.tile([128, HC, BL], BF16, tag="srep16")
            nc.vector.tensor_tensor(out=srep16[:], in0=seg[:],
                                    in1=bcf(rec[:], HC), op=OP.mult)
            return srep16

        with tc.tile_pool(name="act", bufs=3) as ap_, \
             tc.tile_pool(name="actp", bufs=2, space="PSUM") as app:
            with tc.For_i(0, t_steps // U) as iv:
                for u in range(U):
                    t4 = iv * (U * BL) + u * BL
                    tloc = iv * U + u
                    hp_ = p128[:, ds(t4, BL)]
                    t1 = ap_.tile([128, HC, BL], F32, tag="t1")
                    nc.vector.tensor_tensor(
                        out=t1[:], in0=gru[:, ds(tloc, 1), :, :].squeeze(1),
                        in1=bcf(hp_, HC), op=OP.mult)
                    nc.vector.tensor_tensor(out=seg[:], in0=seg[:], in1=t1[:],
                                            op=OP.add)
                    nc.vector.tensor_tensor(out=ha[:], in0=ha[:], in1=hp_, op=OP.add)
                    ge = ap_.tile([128, BL], F32, tag="ge")
                    nc.vector.tensor_single_scalar(out=ge[:], in_=ha[:], scalar=1.0,
                                                   op=OP.is_ge)
                    fire = ap_.tile([128, BL], F32, tag="fire")
                    nc.vector.tensor_tensor(out=fire[:], in0=ge[:],
                                            in1=valid[:, ds(t4, BL)], op=OP.mult)
                    srep16 = seg_repr16(ap_)
                    cnew = ctrl_update_step(ap_, app, srep16)
                    fireH = ap_.tile([128, HC, BL], mybir.dt.uint8, tag="fireH")
                    nc.vector.tensor_copy(fireH[:], bcf(fire[:], HC))
                    nc.vector.copy_predicated(out=ctrl16[:],
                                              mask=fireH[:, 0:CC, :],
                                              data=cnew[:])
                    nc.vector.copy_predicated(out=seg[:], mask=fireH[:],
                                              data=zero_hb[:])
                    nc.vector.tensor_tensor(out=ha[:], in0=ha[:], in1=fire[:],
                                            op=OP.subtract)

            # final remainder update
            rem = ap_.tile([128, BL], F32, tag="rem")
            nc.vector.tensor_single_scalar(out=rem[:], in_=ha[:], scalar=0.01,
                                           op=OP.is_gt)
            srep16 = seg_repr16(ap_)
            cnew = ctrl_update_step(ap_, app, srep16)
            remC = ap_.tile([128, CC, BL], mybir.dt.uint8, tag="remC")
            nc.vector.tensor_copy(remC[:], bcf(rem[:], CC))
            nc.vector.copy_predicated(out=ctrl16[:], mask=remC[:], data=cnew[:])

        # ---- phase I: head ----
        with tc.tile_pool(name="head", bufs=2) as hd, \
             tc.tile_pool(name="headp", bufs=2, space="PSUM") as hdp:
            # gate = sigmoid(gate_w . [h_T, c_T] + gate_b), replicated M=128
            pg = hdp.tile([128, BL], F32, tag="pg")
            for k in range(HC):
                nc.tensor.matmul(pg[:], gatew[:, k * 128:(k + 1) * 128],
                                 h16[:, k, :], start=(k == 0), stop=False)
            for k in range(CC):
                nc.tensor.matmul(pg[:], gatew[:, (HC + k) * 128:(HC + k + 1) * 128],
                                 ctrl16[:, k, :], start=False, stop=(k == CC - 1))
            gate = hd.tile([128, BL], F32, tag="gate")
            nc.scalar.activation(gate[:], pg[:], AF.Sigmoid, bias=gateb_ap[:])
            # co = co_w @ c_T + co_b  (transposed [H, BL])
            pco = hdp.tile([128, HC, BL], F32, tag="pco")
            for m in range(HC):
                for k in range(CC):
                    nc.tensor.matmul(
                        pco[:, m, :], cow[:, k * H + m * 128: k * H + (m + 1) * 128],
                        ctrl16[:, k, :], start=(k == 0), stop=(k == CC - 1))
            cot = hd.tile([128, HC, BL], F32, tag="cot")
            for m in range(HC):
                nc.vector.tensor_scalar_add(cot[:, m, :], pco[:, m, :],
                                            cob[:, m:m + 1])
            # fused = h_T + gate * co
            gco = hd.tile([128, HC, BL], F32, tag="gco")
            nc.vector.tensor_tensor(out=gco[:], in0=cot[:], in1=bcf(gate[:], HC),
                                    op=OP.mult)
            fused16 = hd.tile([128, HC, BL], BF16, tag="fused16")
            nc.vector.tensor_tensor(out=fused16[:], in0=gco[:], in1=h16[:], op=OP.add)
            # hdn = relu(head_w1 @ fused + b1)
            ph1 = hdp.tile([128, HC, BL], F32, tag="ph1")
            for m in range(HC):
                for k in range(HC):
                    nc.tensor.matmul(
                        ph1[:, m, :], hw1[:, k * H + m * 128: k * H + (m + 1) * 128],
                        fused16[:, k, :], start=(k == 0), stop=(k == HC - 1))
            hdn16 = hd.tile([128, HC, BL], BF16, tag="hdn16")
            for m in range(HC):
                nc.scalar.activation(hdn16[:, m, :], ph1[:, m, :], AF.Relu,
                                     bias=hb1[:, m:m + 1])
            # logits = head_w2 @ hdn + b2
            ph2 = hdp.tile([128, NCLS // 128, BL], F32, tag="ph2")
            for m in range(NCLS // 128):
                for k in range(HC):
                    nc.tensor.matmul(
                        ph2[:, m, :], hw2[:, k * NCLS + m * 128: k * NCLS + (m + 1) * 128],
                        hdn16[:, k, :], start=(k == 0), stop=(k == HC - 1))
            lout = hd.tile([128, NCLS // 128, BL], F32, tag="lout")
            for m in range(NCLS // 128):
                nc.vector.tensor_scalar_add(lout[:, m, :], ph2[:, m, :],
                                            hb2[:, m:m + 1])
            # DMA out transposed: out[b, m*128+p] = lout[p, m, b]
            for m in range(NCLS // 128):
                nc.sync.dma_start(
                    bass.AP(out_d.tensor, m * 128, [[1, 128], [NCLS, BL]]),
                    lout[:, m, :])

            if debug:
                hflat = hd.tile([128, 16], F32, tag="hflat")
                nc.vector.tensor_copy(
                    hflat[:], h16[:].rearrange("p a b -> p (a b)"))
                nc.sync.dma_start(dbg_hT_d[:], hflat[:])
                nc.sync.dma_start(dbg_p_d[:], p128[:])
                cflat = hd.tile([128, 8], F32, tag="cflat")
                nc.vector.tensor_copy(
                    cflat[:], ctrl16[:].rearrange("p a b -> p (a b)"))
                nc.sync.dma_start(dbg_ctrl_d[:], cflat[:])
                oflat = hd.tile([128, 16], F32, tag="oflat")
                nc.vector.tensor_copy(
                    oflat[:], out0[:, T - 1, :, :].rearrange("p a b -> p (a b)"))
                nc.sync.dma_start(dbg_o0_d[:], oflat[:])
                nc.sync.dma_start(dbg_ha_d[:], ha[:])

    nc.compile()
    return nc


# ---------------- host side ----------------

def _to_bf16(x):
    return np.ascontiguousarray(x.astype(ml_dtypes.bfloat16))


def _chunked_T(w):
    """w [out_dim, in_dim] -> lhsT sbuf layout [128, (in_dim/128) * out_dim]
    X[p, k*out_dim + j] = w[j, k*128 + p]"""
    out_dim, in_dim = w.shape
    kc = in_dim // 128
    wt = w.T.reshape(kc, 128, out_dim)          # [k, p, j]
    return np.ascontiguousarray(wt.reshape(kc * 128, out_dim)
                                .reshape(kc, 128, out_dim)
                                .transpose(1, 0, 2).reshape(128, kc * out_dim))


def prep_inputs(inputs):
    f32 = np.float32
    ids = np.asarray(inputs["input_ids"]).astype(np.int32)       # [B, T]
    lens = np.asarray(inputs["lengths"]).astype(np.int32)        # [B]
    emb = np.asarray(inputs["emb"], f32)

    def gv(k):
        return np.asarray(inputs[k], f32)

    w_ih0, w_hh0 = gv("w_ih0"), gv("w_hh0")
    b_ih0, b_hh0 = gv("b_ih0"), gv("b_hh0")
    w_ih1, w_hh1 = gv("w_ih1"), gv("w_hh1")
    b_ih1, b_hh1 = gv("b_ih1"), gv("b_hh1")
    bs_w, bs_b = gv("bs_w"), gv("bs_b")
    sp_w, sp_b = gv("sp_w"), gv("sp_b")
    cw_ih, cw_hh = gv("ctrl_w_ih"), gv("ctrl_w_hh")
    cb_ih, cb_hh = gv("ctrl_b_ih"), gv("ctrl_b_hh")
    co_w, co_b = gv("co_w"), gv("co_b")
    gate_w, gate_b = gv("gate_w"), gv("gate_b")
    hw1_, hb1_ = gv("head_w1"), gv("head_b1")
    hw2_, hb2_ = gv("head_w2"), gv("head_b2")

    def bias_sb(vec, ncols):
        return np.ascontiguousarray(vec.reshape(ncols, 128).T.astype(f32))

    bias0 = np.concatenate([(b_ih0 + b_hh0)[:2 * H], b_ih0[2 * H:]])
    bias1 = np.concatenate([(b_ih1 + b_hh1)[:2 * H], b_ih1[2 * H:]])
    b_gi_eff = cw_ih @ sp_b + cb_ih
    cbiasA = np.concatenate([b_gi_eff[:2 * C] + cb_hh[:2 * C], b_gi_eff[2 * C:]])
    cbias = np.concatenate([cbiasA, cb_hh[2 * C:]]).reshape(1, 1024)

    shared = {
        "emb": emb,
        "wih0T": _to_bf16(_chunked_T(w_ih0)),
        "whh0T": _to_bf16(_chunked_T(w_hh0)),
        "wih1T": _to_bf16(_chunked_T(w_ih1)),
        "whh1T": _to_bf16(_chunked_T(w_hh1)),
        "bias0": bias_sb(bias0, GC), "bias1": bias_sb(bias1, GC),
        "bhn0r": _to_bf16(b_hh0[2 * H:].reshape(1, H)),
        "bhn1r": _to_bf16(b_hh1[2 * H:].reshape(1, H)),
        "bswT": _to_bf16(np.repeat(bs_w[0].reshape(HC, 128).transpose(1, 0)
                                   [:, :, None], 128, axis=2).reshape(128, HC * 128)),
        "spwT": _to_bf16(_chunked_T(sp_w)),
        "cwihT": _to_bf16(_chunked_T(cw_ih)),
        "cwhhT": _to_bf16(_chunked_T(cw_hh)),
        "cbias": _to_bf16(cbias),
        "cowT": _to_bf16(_chunked_T(co_w)),
        "cob": bias_sb(co_b, HC),
        "gatewT": _to_bf16(np.repeat(gate_w[0].reshape(HC + CC, 128).transpose(1, 0)
                                     [:, :, None], 128, axis=2)
                           .reshape(128, (HC + CC) * 128)),
        "hw1T": _to_bf16(_chunked_T(hw1_)),
        "hb1": bias_sb(hb1_, HC),
        "hw2T": _to_bf16(_chunked_T(hw2_)),
        "hb2": bias_sb(hb2_, NCLS // 128),
    }
    in_maps = []
    for c in range(NCORES):
        lanes = slice(c * BL, (c + 1) * BL)
        ids_lin = ids[lanes].T.reshape(NT)            # [t*BL + b]
        ids_sb = ids_lin.reshape(NT // 128, 128).T    # [p, g]
        lens_rep = np.broadcast_to(lens[lanes].astype(f32), (128, BL))
        m = dict(shared)
        m["ids"] = np.ascontiguousarray(ids_sb)
        m["lens"] = np.ascontiguousarray(lens_rep)
        in_maps.append(m)
    scalars = {"bs_b": float(bs_b[0]), "gate_b": float(gate_b[0])}
    return in_maps, scalars


_CACHE = {}


def run(inputs, trace=False, t_steps=T, debug=False):
    in_maps, scalars = prep_inputs(inputs)
    key = (t_steps, debug)
    if key not in _CACHE:
        _CACHE[key] = build(scalars, t_steps=t_steps, debug=debug)
    nc = _CACHE[key]
    res = run_bass_kernel_spmd(nc, in_maps, core_ids=list(range(NCORES)),
                               trace=trace)
    out = np.concatenate([res.results[c]["out"] for c in range(NCORES)], axis=0)
    return out.astype(np.float32), res


def kernel(**inputs):
    out, _ = run(inputs, trace=False)
    return out
